# revision 1
# baseline (speedup 1.0000x reference)
"""NT-Xent loss on 8 Trainium2 NeuronCores.

Math (reference): xn = row-normalized x; mat = exp(xn @ xn.T / 0.1) with zero
diagonal; numer_r = mat[r, r±B]; denom_r = column sum r; loss = -mean(log(numer/denom)).

Because mat is symmetric, column sums equal row sums, so a core that owns a
row block [1024, 8192] computes its denominators entirely locally — no
collectives.  Each core c receives x rolled by -1024*c rows so that, in its
local column coordinates, the diagonal sits at col j'=i and the positive pair
at col j'=4096+i for local row i: the special tiles are at the same
compile-time position on every core, keeping the program SPMD-uniform.

Per-core pipeline:
  1. Stream x row-tiles [128,512]; ACT Square+accum row-sum; DVE
     reciprocal + ACT sqrt -> 1/norm; DVE per-partition scale -> xn (bf16).
  2. Transpose xn via PE matmul against identity (out = xn_tile.T @ I),
     PSUM->SBUF copy on DVE -> xnT tiles ([128(d),*] bf16).  Columns 0..1023
     of xnT double as the stationary (lhsT) operand.
  3. For each 1024-wide column pair: matmul accumulate over d (4x128) into
     PSUM [128,1024]; one ACT Exp(scale=10) pass PSUM->SBUF with accum_out
     giving the row-sum; on pair 0 / pair 4 extract the diagonal / positive
     values with a fused DVE multiply-by-identity reduce.
  4. denom = rowsum - diag.  Host applies log and the final mean.
"""

import functools

import ml_dtypes
import numpy as np

N, D, B = 8192, 512, 4096
NCORES = 8
RPC = N // NCORES           # 1024 local rows per core
MB = RPC // 128             # 8 row blocks of 128
NT = N // 128               # 64 row tiles of x
KT = D // 128               # 4 contraction subtiles
PAIRS = N // 1024           # 8 column pairs
NUMER_PAIR = B // 1024      # positive pair lands in column pair 4
TEMP_INV = 10.0             # 1 / temperature


def _build():
    from contextlib import ExitStack

    import concourse.bacc as bacc
    import concourse.mybir as mybir
    import concourse.tile as tile

    F32 = mybir.dt.float32
    BF16 = mybir.dt.bfloat16
    I32 = mybir.dt.int32
    ALU = mybir.AluOpType
    ACTF = mybir.ActivationFunctionType
    AX = mybir.AxisListType

    nc = bacc.Bacc("TRN2", target_bir_lowering=False, debug=False,
                   num_devices=NCORES)
    x_in = nc.dram_tensor("x", [N, D], F32, kind="ExternalInput").ap()
    eye16_in = nc.dram_tensor("eye16", [128, 128], BF16, kind="ExternalInput").ap()
    eye32_in = nc.dram_tensor("eye32", [128, 128], F32, kind="ExternalInput").ap()
    numer_out = nc.dram_tensor("numer", [128, MB], F32, kind="ExternalOutput").ap()
    denom_out = nc.dram_tensor("denom", [128, MB], F32, kind="ExternalOutput").ap()

    with ExitStack() as ctx:
        tc = ctx.enter_context(tile.TileContext(nc))
        consts = ctx.enter_context(tc.tile_pool(name="consts", bufs=1))
        dtp = ctx.enter_context(tc.tile_pool(name="dtp", bufs=1))
        junkp = ctx.enter_context(tc.tile_pool(name="junk", bufs=2))
        xnp = ctx.enter_context(tc.tile_pool(name="xn", bufs=1))
        stats = ctx.enter_context(tc.tile_pool(name="stats", bufs=1))
        lhsp = ctx.enter_context(tc.tile_pool(name="lhs", bufs=1))
        rhsp = ctx.enter_context(tc.tile_pool(name="rhs", bufs=2))
        expp = ctx.enter_context(tc.tile_pool(name="expo", bufs=4))
        pst = ctx.enter_context(tc.tile_pool(name="pst", bufs=3, space="PSUM"))
        psm = ctx.enter_context(tc.tile_pool(name="psm", bufs=2, space="PSUM"))

        eye16 = consts.tile([128, 128], BF16, tag="eye16")
        nc.sync.dma_start(eye16[:], eye16_in)
        eye32 = consts.tile([128, 128], F32, tag="eye32")
        nc.sync.dma_start(eye32[:], eye32_in)

        ss = stats.tile([128, NT], F32, tag="ss")
        invn = stats.tile([128, NT], F32, tag="invn")
        rs = stats.tile([128, MB * PAIRS], F32, tag="rs")
        diagv = stats.tile([128, MB], F32, tag="diagv")
        numv = stats.tile([128, MB], F32, tag="numv")
        rowsum = stats.tile([128, MB], F32, tag="rowsum")
        dent = stats.tile([128, MB], F32, tag="dent")

        xn = [xnp.tile([128, D], BF16, tag=f"xn{i}", name=f"xn{i}")
              for i in range(NT)]
        dts = [dtp.tile([128, 128], BF16, tag=f"dt{i}", name=f"dt{i}")
               for i in range(NT)]

        # Newton-rsqrt scratch (int bit-trick seed, 3 iterations)
        iu = stats.tile([128, NT], I32, tag="iu")
        iv = stats.tile([128, NT], I32, tag="iv")
        nt_t = stats.tile([128, NT], F32, tag="nt_t")

        # Normalize-group emitter: bf16 cast-loads, row sum-of-squares,
        # Newton rsqrt (bit-trick seed), diag(inv) tiles.
        def emit_group(gstart, gsz):
            assert gsz % 4 == 0
            for q in range(gstart // 4, (gstart + gsz) // 4):
                jb = junkp.tile([128, 4 * D], BF16, tag="sqj", name="jb")
                for j in range(4):
                    i = 4 * q + j
                    nc.gpsimd.dma_start(xn[i][:], x_in[i * 128:(i + 1) * 128, :])
                    nc.vector.tensor_mul(jb[:, j * D:(j + 1) * D],
                                         xn[i][:], xn[i][:])
                nc.vector.tensor_reduce(
                    ss[:, 4 * q:4 * q + 4],
                    jb[:].rearrange("p (a b) -> p a b", a=4),
                    axis=AX.X, op=ALU.add)
            sl = slice(gstart, gstart + gsz)
            nc.vector.tensor_scalar(iu[:, sl], ss[:, sl].bitcast(I32), 1, None,
                                    op0=ALU.arith_shift_right)
            nc.vector.tensor_scalar(iv[:, sl], iu[:, sl], -1, 0x5F3759DF,
                                    op0=ALU.mult, op1=ALU.add)
            y = iv[:, sl].bitcast(F32)
            for it in range(3):
                nc.vector.tensor_mul(nt_t[:, sl], y, y)
                nc.vector.tensor_mul(nt_t[:, sl], nt_t[:, sl], ss[:, sl])
                nc.vector.tensor_scalar(nt_t[:, sl], nt_t[:, sl], -0.5, 1.5,
                                        op0=ALU.mult, op1=ALU.add)
                out_y = invn[:, sl] if it == 2 else y
                nc.vector.tensor_mul(out_y, y, nt_t[:, sl])
            for i in range(gstart, gstart + gsz):
                nc.vector.tensor_scalar_mul(dts[i][:], eye16[:],
                                            invn[:, i:i + 1])

        # (gstart, gsz) groups; pair tp consumes tiles [8tp, 8tp+8)
        groups = [(0, 4), (4, 4)] + [(8 * g, 8) for g in range(1, 8)]
        emit_group(*groups[0])
        emit_group(*groups[1])

        lhs = [lhsp.tile([128, RPC], BF16, tag=f"lhs{k}", name=f"lhs{k}")
               for k in range(KT)]

        # Phases 2+3 interleaved per column pair; emit the normalize group
        # feeding pair tp+2 right before pair tp so the DVE queue interleaves.
        for tp in range(PAIRS):
            if tp + 2 < len(groups):
                emit_group(*groups[tp + 2])
            rhs = lhs if tp == 0 else [
                rhsp.tile([128, 1024], BF16, tag=f"rhs{k}", name=f"rhs{k}")
                for k in range(KT)]
            for h in range(2):
                t = 2 * tp + h
                for k in range(KT):
                    ps = pst.tile([128, 512], F32, tag="pst")
                    for j in range(4):
                        nc.tensor.matmul(
                            ps[:, j * 128:(j + 1) * 128],
                            lhsT=xn[4 * t + j][:, k * 128:(k + 1) * 128],
                            rhs=dts[4 * t + j][:], start=True, stop=True)
                    dst = rhs[k][:, h * 512:(h + 1) * 512]
                    if h == 0:
                        nc.scalar.copy(dst, ps[:])
                    else:
                        nc.vector.tensor_copy(dst, ps[:])
            for m in range(MB):
                ps = psm.tile([128, 1024], F32, tag="psm")
                for k in range(KT):
                    for h in range(2):
                        nc.tensor.matmul(
                            ps[:, h * 512:(h + 1) * 512],
                            lhsT=lhs[k][:, m * 128:(m + 1) * 128],
                            rhs=rhs[k][:, h * 512:(h + 1) * 512],
                            start=(k == 0), stop=(k == KT - 1))
                eo = expp.tile([128, 1024], F32, tag="eo")
                col = m * PAIRS + tp
                nc.scalar.activation(eo[:], ps[:], ACTF.Exp, scale=TEMP_INV,
                                     accum_out=rs[:, col:col + 1])
                if tp == 0 or tp == NUMER_PAIR:
                    tgt = diagv if tp == 0 else numv
                    junk = junkp.tile([128, 128], F32, tag="ttj")
                    nc.vector.tensor_mul(junk[:], eo[:, m * 128:(m + 1) * 128],
                                         eye32[:])
                    nc.vector.tensor_reduce(tgt[:, m:m + 1], junk[:],
                                            axis=AX.X, op=ALU.add)

        # Finalize: denom = full row sum - diagonal term.
        for m in range(MB):
            nc.vector.tensor_reduce(rowsum[:, m:m + 1],
                                    rs[:, m * PAIRS:(m + 1) * PAIRS],
                                    axis=AX.X, op=ALU.add)
        nc.vector.tensor_sub(dent[:], rowsum[:], diagv[:])
        nc.sync.dma_start(numer_out, numv[:])
        nc.sync.dma_start(denom_out, dent[:])

    nc.finalize()
    return nc


@functools.lru_cache(maxsize=1)
def _get_nc():
    return _build()


def _run(x, **run_kwargs):
    from concourse.bass_utils import run_bass_kernel_spmd

    x = np.ascontiguousarray(np.asarray(x), dtype=np.float32)
    assert x.shape == (N, D)
    eye16 = np.eye(128, dtype=ml_dtypes.bfloat16)
    eye32 = np.eye(128, dtype=np.float32)
    in_maps = [
        {"x": np.ascontiguousarray(np.roll(x, -c * RPC, axis=0)),
         "eye16": eye16, "eye32": eye32}
        for c in range(NCORES)
    ]
    nc = _get_nc()
    return run_bass_kernel_spmd(nc, in_maps, list(range(NCORES)), **run_kwargs)


def _loss_from_results(results):
    num = np.concatenate(
        [results[c]["numer"].T.reshape(-1) for c in range(NCORES)])
    den = np.concatenate(
        [results[c]["denom"].T.reshape(-1) for c in range(NCORES)])
    loss = -np.sum(np.log(num.astype(np.float64) / den.astype(np.float64))) / N
    return np.float32(loss)


def kernel(x):
    res = _run(x)
    return _loss_from_results(res.results)



# revision 7
# speedup vs baseline: 1.6025x; 1.6025x over previous
"""NT-Xent loss on 8 Trainium2 NeuronCores — triangular fp8 scheme.

Math: xn = row-normalized x; mat = exp(xn @ xn.T / 0.1) with zero diag;
numer_r = mat[r, (r+B) mod N]; denom_r = column sum r (= row sum r, mat
symmetric); loss = -mean(log(numer/denom)).

Work assignment (circulant triangle): core c owns row block c (1024 rows,
input rolled by -1024c so everything is SPMD-uniform) and computes only
column pairs j = 0..4 (local cols 0..5119), i.e. blocks (c, c+j mod 8).
Row block b then recovers its full denominator from
  - its own row sums over pairs 0..4      (cols b..b+4)
  - COLUMN sums of blocks (b-j, b), j=1..3, computed on cores b-1..b-3
    as the column sums of their pairs 1..3 (mat symmetry).
Pair 4 (c, c+4) is computed redundantly by both partner cores so no
colsum exchange is needed for it. The colsum partials ([128,1024] per
pair, partition-summed on the host) plus own rowsums are combined on the
host, which already does the final log/mean — no device collectives.

Precision: operands are fp8 e4m3 (xn * 16), matmul accumulates fp32 in
PSUM via DoubleRow perf mode (2 k-tiles per instruction, 2x PE rate),
exp runs on ACT straight from PSUM with scale 10/256 and accum_out row
sums. diag/positive-pair values are extracted pre-exp from PSUM at fp32
(fused DVE tensor_tensor_reduce against an identity), exponentiated in
one tiny ACT op — the diag subtraction therefore cancels exactly and
fp8 quantization noise averages out across 8192 rows (< 1e-3 rel).

Per-core pipeline:
  1. DMA 5 x-tile groups [128, 8x512] bf16 (host pre-rolls + casts).
  2. DVE tensor_tensor_reduce -> row sum-of-squares; Newton rsqrt
     (bit-trick seed) -> invn*16 (bf16); one broadcast multiply builds
     all diag(invn*16) tiles.
  3. PE transposes xn tiles against the diag tiles -> PSUM [128,2048]
     fills; ACT/DVE drain-cast them to fp8 xt tiles (layout
     [d-slice k*1024 + col], giving DoubleRow APs by pure slicing).
  4. Mains: per (m, pair-group) fp8 DoubleRow matmuls accumulate
     [128, 1024] per pair in PSUM; one ACT Exp per fill with accum_out
     rowsum; DVE accumulates colsums for pairs 1..3 from the bf16 exp
     tiles; fused DVE extracts sdiag/spos from PSUM on pairs 0/4.
  5. dent = rowsum - exp(sdiag); numer = exp(spos); DMA out with the
     three colsum partials.
"""

import functools

import ml_dtypes
import numpy as np

N, D, B = 8192, 512, 4096
NCORES = 8
RPC = N // NCORES           # 1024 rows per core
PAIRS = 5                   # column pairs computed per core
TILES = 8 * PAIRS           # 40 row tiles of rolled x
ROWS_IN = TILES * 128       # 5120 input rows per core
MB = RPC // 128             # 8 row blocks of 128
SCALE = 16.0                # fp8 operand scale
EXPS = 10.0 / (SCALE * SCALE)  # activation scale: 1/temp / SCALE^2
PGROUPS = ((0, 1), (2, 3), (4,))


def _build():
    from contextlib import ExitStack

    import concourse.bacc as bacc
    import concourse.mybir as mybir
    import concourse.tile as tile

    F32 = mybir.dt.float32
    BF16 = mybir.dt.bfloat16
    FP8 = mybir.dt.float8e4
    I32 = mybir.dt.int32
    ALU = mybir.AluOpType
    ACTF = mybir.ActivationFunctionType
    AX = mybir.AxisListType
    DR = mybir.MatmulPerfMode.DoubleRow

    nc = bacc.Bacc("TRN2", target_bir_lowering=False, debug=False,
                   num_devices=NCORES)
    x_in = nc.dram_tensor("x", [ROWS_IN, D], BF16, kind="ExternalInput").ap()
    eye16_in = nc.dram_tensor("eye16", [128, 128], BF16, kind="ExternalInput").ap()
    eye32_in = nc.dram_tensor("eye32", [128, 128], F32, kind="ExternalInput").ap()
    numer_out = nc.dram_tensor("numer", [128, MB], F32, kind="ExternalOutput").ap()
    denom_out = nc.dram_tensor("denom", [128, MB], F32, kind="ExternalOutput").ap()
    colsum_out = nc.dram_tensor("colsum", [128, 3 * RPC], BF16,
                                kind="ExternalOutput").ap()

    with ExitStack() as ctx:
        tc = ctx.enter_context(tile.TileContext(nc))
        consts = ctx.enter_context(tc.tile_pool(name="consts", bufs=1))
        xldp = ctx.enter_context(tc.tile_pool(name="xld", bufs=1))
        stats = ctx.enter_context(tc.tile_pool(name="stats", bufs=1))
        xtp = ctx.enter_context(tc.tile_pool(name="xt", bufs=1))
        eop = ctx.enter_context(tc.tile_pool(name="eo", bufs=2))
        colp = ctx.enter_context(tc.tile_pool(name="col", bufs=1))
        junkp = ctx.enter_context(tc.tile_pool(name="junk", bufs=2))
        psm = ctx.enter_context(tc.tile_pool(name="psm", bufs=2, space="PSUM"))

        eye16 = consts.tile([128, 128], BF16, tag="eye16")
        nc.sync.dma_start(eye16[:], eye16_in)
        eye32 = consts.tile([128, 128], F32, tag="eye32")
        nc.sync.dma_start(eye32[:], eye32_in)

        xld = [xldp.tile([128, 8 * D], BF16, tag=f"xld{g}", name=f"xld{g}")
               for g in range(PAIRS)]

        def xn(t):  # [128, 512] view of row tile t
            return xld[t // 8][:, (t % 8) * D:(t % 8 + 1) * D]

        ss = stats.tile([128, TILES], F32, tag="ss")
        invn = stats.tile([128, TILES], F32, tag="invn")
        invn16 = stats.tile([128, TILES], BF16, tag="invn16")
        iu = stats.tile([128, TILES], I32, tag="iu")
        iv = stats.tile([128, TILES], I32, tag="iv")
        nt_t = stats.tile([128, TILES], F32, tag="nt_t")
        dts = stats.tile([128, TILES * 128], BF16, tag="dts")
        rs = stats.tile([128, MB * 3], F32, tag="rs")
        sdiag = stats.tile([128, MB], F32, tag="sdiag")
        spos = stats.tile([128, MB], F32, tag="spos")
        diagexp = stats.tile([128, MB], F32, tag="diagexp")
        numv = stats.tile([128, MB], F32, tag="numv")
        rowsum = stats.tile([128, MB], F32, tag="rowsum")
        dent = stats.tile([128, MB], F32, tag="dent")

        xt = [xtp.tile([128, 4 * 1024], FP8, tag=f"xt{j}", name=f"xt{j}")
              for j in range(PAIRS)]
        colacc = [colp.tile([128, RPC], BF16, tag=f"col{j}", name=f"col{j}")
                  for j in range(3)]

        # Input loads: one DMA per 8-tile group; tile t row r lives at
        # x[t*128 + p, :] -> dst [128, (8, 512)].
        for g in range(PAIRS):
            src = x_in[g * 1024:(g + 1) * 1024, :].rearrange(
                "(s p) d -> p s d", p=128)
            nc.sync.dma_start(xld[g][:].rearrange("p (s d) -> p s d", s=8), src)

        def norm_group(g):
            """Row sum-of-squares, Newton rsqrt (*16, bf16), diag tiles."""
            for q in range(8):
                t = 8 * g + q
                jb = junkp.tile([128, D], BF16, tag="nj", name="nj")
                nc.vector.scalar_tensor_tensor(
                    jb[:], xn(t), 1.0, xn(t),
                    op0=ALU.mult, op1=ALU.mult, accum_out=ss[:, t:t + 1])
            sl = slice(8 * g, 8 * g + 8)
            nc.vector.tensor_scalar(iu[:, sl], ss[:, sl].bitcast(I32), 1, None,
                                    op0=ALU.arith_shift_right)
            nc.vector.tensor_scalar(iv[:, sl], iu[:, sl], -1, 0x5F3759DF,
                                    op0=ALU.mult, op1=ALU.add)
            y = iv[:, sl].bitcast(F32)
            for it in range(3):
                nc.vector.tensor_mul(nt_t[:, sl], y, y)
                nc.vector.tensor_mul(nt_t[:, sl], nt_t[:, sl], ss[:, sl])
                nc.vector.tensor_scalar(nt_t[:, sl], nt_t[:, sl], -0.5, 1.5,
                                        op0=ALU.mult, op1=ALU.add)
                out_y = invn[:, sl] if it == 2 else y
                nc.vector.tensor_mul(out_y, y, nt_t[:, sl])
            nc.vector.tensor_scalar(invn16[:, sl], invn[:, sl], SCALE, None,
                                    op0=ALU.mult)
            # dts[:, t*128+c] = (p==c) * invn16[p, t] for the group's tiles
            nc.vector.tensor_tensor(
                dts[:, 8 * g * 128:(8 * g + 8) * 128].rearrange(
                    "p (s c) -> p s c", s=8),
                eye16[:, None, :].to_broadcast([128, 8, 128]),
                invn16[:, sl, None].to_broadcast([128, 8, 128]),
                op=ALU.mult)

        def transpose_pair(j):
            """xt[j][:, k*1024 + tt*128 + r] = xn(8j+tt)[r, k*128+d] * invn16*16."""
            for kk in range(2):
                ps = psm.tile([128, 2048], F32, tag="ps", name=f"tf{j}_{kk}")
                for tt in range(8):
                    t = 8 * j + tt
                    for ks in range(2):
                        k = 2 * kk + ks
                        nc.tensor.matmul(
                            ps[:, ks * 1024 + tt * 128:ks * 1024 + (tt + 1) * 128],
                            lhsT=xn(t)[:, k * 128:(k + 1) * 128],
                            rhs=dts[:, t * 128:(t + 1) * 128],
                            start=True, stop=True)
                dst = xt[j][:, kk * 2048:(kk + 1) * 2048]
                if kk == 0:
                    nc.scalar.copy(dst, ps[:])
                else:
                    nc.vector.tensor_copy(dst, ps[:])

        def dr_ap(j, kk, lo, w):
            """[128, 2, w] DoubleRow AP over xt[j], k-slices {2kk, 2kk+1}."""
            return xt[j][:, kk * 2048:(kk + 1) * 2048].rearrange(
                "p (two c) -> p two c", two=2)[:, :, lo:lo + w]

        def main_fill(m, pgi):
            pg = PGROUPS[pgi]
            w = 1024 * len(pg)
            ps = psm.tile([128, 2048], F32, tag="ps", name=f"mf{m}_{pgi}")
            for ji, j in enumerate(pg):
                for h in range(2):
                    for kk in range(2):
                        nc.tensor.matmul(
                            ps[:, ji * 1024 + h * 512:ji * 1024 + (h + 1) * 512],
                            lhsT=dr_ap(0, kk, m * 128, 128),
                            rhs=dr_ap(j, kk, h * 512, 512),
                            start=(kk == 0), stop=(kk == 1),
                            perf_mode=DR)
            # pre-exp extracts from PSUM (exact fp32): self-diag / positive
            if pgi == 0 or pgi == 2:
                tgt = sdiag if pgi == 0 else spos
                junk = junkp.tile([128, 128], F32, tag="xj", name="xj")
                nc.vector.scalar_tensor_tensor(
                    junk[:], ps[:, m * 128:(m + 1) * 128], 1.0, eye32[:],
                    op0=ALU.mult, op1=ALU.mult, accum_out=tgt[:, m:m + 1])
            eo = eop.tile([128, 2048], BF16, tag="eo", name=f"eo{m}_{pgi}")
            nc.scalar.activation(eo[:, :w], ps[:, :w], ACTF.Exp, scale=EXPS,
                                 accum_out=rs[:, m * 3 + pgi:m * 3 + pgi + 1])
            for ji, j in enumerate(pg):
                if j in (1, 2, 3):
                    src = eo[:, ji * 1024:(ji + 1) * 1024]
                    if m == 0:
                        nc.vector.tensor_copy(colacc[j - 1][:], src)
                    else:
                        nc.vector.tensor_add(colacc[j - 1][:],
                                             colacc[j - 1][:], src)

        # ---- emission schedule (pipelined) ----
        norm_group(0)
        norm_group(1)
        transpose_pair(0)
        transpose_pair(1)
        for m in range(3):
            main_fill(m, 0)
        norm_group(2)
        transpose_pair(2)
        for m in range(3, 6):
            main_fill(m, 0)
        norm_group(3)
        transpose_pair(3)
        for m in range(6, 8):
            main_fill(m, 0)
        for m in range(3):
            main_fill(m, 1)
        norm_group(4)
        transpose_pair(4)
        for m in range(3, 8):
            main_fill(m, 1)
        for m in range(8):
            main_fill(m, 2)

        # ---- finalize ----
        nc.vector.tensor_reduce(rowsum[:], rs[:].rearrange("p (m q) -> p m q", m=MB),
                                axis=AX.X, op=ALU.add)
        nc.scalar.activation(diagexp[:], sdiag[:], ACTF.Exp, scale=EXPS)
        nc.scalar.activation(numv[:], spos[:], ACTF.Exp, scale=EXPS)
        nc.vector.tensor_sub(dent[:], rowsum[:], diagexp[:])
        nc.sync.dma_start(numer_out, numv[:])
        nc.sync.dma_start(denom_out, dent[:])
        for j in range(3):
            nc.sync.dma_start(colsum_out[:, j * RPC:(j + 1) * RPC], colacc[j][:])

    nc.finalize()
    return nc


@functools.lru_cache(maxsize=1)
def _get_nc():
    return _build()


def _in_maps(x):
    x = np.asarray(x)
    assert x.shape == (N, D)
    xb = np.asarray(x, dtype=np.float32).astype(ml_dtypes.bfloat16)
    eye16 = np.eye(128, dtype=ml_dtypes.bfloat16)
    eye32 = np.eye(128, dtype=np.float32)
    return [
        {"x": np.ascontiguousarray(np.roll(xb, -c * RPC, axis=0)[:ROWS_IN]),
         "eye16": eye16, "eye32": eye32}
        for c in range(NCORES)
    ]


def _run(x, **run_kwargs):
    from concourse.bass_utils import run_bass_kernel_spmd

    nc = _get_nc()
    return run_bass_kernel_spmd(nc, _in_maps(x), list(range(NCORES)),
                                **run_kwargs)


def _loss_from_results(results):
    num = np.concatenate(
        [results[c]["numer"].T.reshape(-1).astype(np.float64)
         for c in range(NCORES)])
    den_own = [results[c]["denom"].T.reshape(-1).astype(np.float64)
               for c in range(NCORES)]
    # colsum[c][:, (j-1)*1024 + cl].sum over partitions = partial denom for
    # global row (c+j)*1024 + cl, j in {1,2,3}
    cs = [np.asarray(results[c]["colsum"], dtype=np.float64)
          for c in range(NCORES)]
    den = np.concatenate([
        den_own[b] + sum(
            cs[(b - j) % NCORES][:, (j - 1) * RPC:j * RPC].sum(axis=0)
            for j in range(1, 4))
        for b in range(NCORES)])
    loss = -np.sum(np.log(num / den)) / N
    return np.float32(loss)


def kernel(x):
    res = _run(x)
    return _loss_from_results(res.results)


# revision 9
# speedup vs baseline: 1.6043x; 1.0011x over previous
"""NT-Xent loss on 8 Trainium2 NeuronCores — triangular fp8 scheme.

Math: xn = row-normalized x; mat = exp(xn @ xn.T / 0.1) with zero diag;
numer_r = mat[r, (r+B) mod N]; denom_r = column sum r (= row sum r, mat
symmetric); loss = -mean(log(numer/denom)).

Work assignment (circulant triangle): core c owns row block c (1024 rows,
input rolled by -1024c so everything is SPMD-uniform) and computes only
column pairs j = 0..4 (local cols 0..5119), i.e. blocks (c, c+j mod 8).
Row block b then recovers its full denominator from
  - its own row sums over pairs 0..4      (cols b..b+4)
  - COLUMN sums of blocks (b-j, b), j=1..3, computed on cores b-1..b-3
    as the column sums of their pairs 1..3 (mat symmetry).
Pair 4 (c, c+4) is computed redundantly by both partner cores so no
colsum exchange is needed for it. The colsum partials ([128,1024] per
pair, partition-summed on the host) plus own rowsums are combined on the
host, which already does the final log/mean — no device collectives.

Precision: operands are fp8 e4m3 (xn * 16), matmul accumulates fp32 in
PSUM via DoubleRow perf mode (2 k-tiles per instruction, 2x PE rate),
exp runs on ACT straight from PSUM with scale 10/256 and accum_out row
sums. diag/positive-pair values are extracted pre-exp from PSUM at fp32
(fused DVE tensor_tensor_reduce against an identity), exponentiated in
one tiny ACT op — the diag subtraction therefore cancels exactly and
fp8 quantization noise averages out across 8192 rows (< 1e-3 rel).

Per-core pipeline:
  1. DMA 5 x-tile groups [128, 8x512] bf16 (host pre-rolls + casts).
  2. DVE tensor_tensor_reduce -> row sum-of-squares; Newton rsqrt
     (bit-trick seed) -> invn*16 (bf16); one broadcast multiply builds
     all diag(invn*16) tiles.
  3. PE transposes xn tiles against the diag tiles -> PSUM [128,2048]
     fills; ACT/DVE drain-cast them to fp8 xt tiles (layout
     [d-slice k*1024 + col], giving DoubleRow APs by pure slicing).
  4. Mains: per (m, pair-group) fp8 DoubleRow matmuls accumulate
     [128, 1024] per pair in PSUM; one ACT Exp per fill with accum_out
     rowsum; DVE accumulates colsums for pairs 1..3 from the bf16 exp
     tiles; fused DVE extracts sdiag/spos from PSUM on pairs 0/4.
  5. dent = rowsum - exp(sdiag); numer = exp(spos); DMA out with the
     three colsum partials.
"""

import functools

import ml_dtypes
import numpy as np

N, D, B = 8192, 512, 4096
NCORES = 8
RPC = N // NCORES           # 1024 rows per core
PAIRS = 5                   # column pairs computed per core
TILES = 8 * PAIRS           # 40 row tiles of rolled x
ROWS_IN = TILES * 128       # 5120 input rows per core
MB = RPC // 128             # 8 row blocks of 128
SCALE = 16.0                # fp8 operand scale
EXPS = 10.0 / (SCALE * SCALE)  # activation scale: 1/temp / SCALE^2
PGROUPS = ((0, 1), (2, 3), (4,))


def _build():
    from contextlib import ExitStack

    import concourse.bacc as bacc
    import concourse.mybir as mybir
    import concourse.tile as tile

    F32 = mybir.dt.float32
    BF16 = mybir.dt.bfloat16
    FP8 = mybir.dt.float8e4
    I32 = mybir.dt.int32
    ALU = mybir.AluOpType
    ACTF = mybir.ActivationFunctionType
    AX = mybir.AxisListType
    DR = mybir.MatmulPerfMode.DoubleRow

    nc = bacc.Bacc("TRN2", target_bir_lowering=False, debug=False,
                   num_devices=NCORES)
    x_in = nc.dram_tensor("x", [ROWS_IN, D], BF16, kind="ExternalInput").ap()
    eye16_in = nc.dram_tensor("eye16", [128, 128], BF16, kind="ExternalInput").ap()
    eye32_in = nc.dram_tensor("eye32", [128, 128], F32, kind="ExternalInput").ap()
    numer_out = nc.dram_tensor("numer", [128, MB], F32, kind="ExternalOutput").ap()
    denom_out = nc.dram_tensor("denom", [128, MB], F32, kind="ExternalOutput").ap()
    colsum_out = nc.dram_tensor("colsum", [128, 3 * RPC], BF16,
                                kind="ExternalOutput").ap()

    with ExitStack() as ctx:
        tc = ctx.enter_context(tile.TileContext(nc))
        consts = ctx.enter_context(tc.tile_pool(name="consts", bufs=1))
        xldp = ctx.enter_context(tc.tile_pool(name="xld", bufs=1))
        stats = ctx.enter_context(tc.tile_pool(name="stats", bufs=1))
        xtp = ctx.enter_context(tc.tile_pool(name="xt", bufs=1))
        eop = ctx.enter_context(tc.tile_pool(name="eo", bufs=4))
        colp = ctx.enter_context(tc.tile_pool(name="col", bufs=1))
        junkp = ctx.enter_context(tc.tile_pool(name="junk", bufs=2))
        psm = ctx.enter_context(tc.tile_pool(name="psm", bufs=2, space="PSUM"))

        eye16 = consts.tile([128, 128], BF16, tag="eye16")
        nc.sync.dma_start(eye16[:], eye16_in)
        eye32 = consts.tile([128, 128], F32, tag="eye32")
        nc.sync.dma_start(eye32[:], eye32_in)

        xld = [xldp.tile([128, 8 * D], BF16, tag=f"xld{g}", name=f"xld{g}")
               for g in range(PAIRS)]

        def xn(t):  # [128, 512] view of row tile t
            return xld[t // 8][:, (t % 8) * D:(t % 8 + 1) * D]

        ss = stats.tile([128, TILES], F32, tag="ss")
        invn = stats.tile([128, TILES], F32, tag="invn")
        invn16 = stats.tile([128, TILES], BF16, tag="invn16")
        iu = stats.tile([128, TILES], I32, tag="iu")
        iv = stats.tile([128, TILES], I32, tag="iv")
        nt_t = stats.tile([128, TILES], F32, tag="nt_t")
        dts = stats.tile([128, TILES * 128], BF16, tag="dts")
        rs = stats.tile([128, MB * 3], F32, tag="rs")
        sdiag = stats.tile([128, MB], F32, tag="sdiag")
        spos = stats.tile([128, MB], F32, tag="spos")
        diagexp = stats.tile([128, MB], F32, tag="diagexp")
        numv = stats.tile([128, MB], F32, tag="numv")
        rowsum = stats.tile([128, MB], F32, tag="rowsum")
        dent = stats.tile([128, MB], F32, tag="dent")

        xt = [xtp.tile([128, 4 * 1024], FP8, tag=f"xt{j}", name=f"xt{j}")
              for j in range(PAIRS)]
        colacc = [colp.tile([128, RPC], BF16, tag=f"col{j}", name=f"col{j}")
                  for j in range(3)]

        # Input loads: one DMA per 8-tile group; tile t row r lives at
        # x[t*128 + p, :] -> dst [128, (8, 512)].
        for g in range(PAIRS):
            src = x_in[g * 1024:(g + 1) * 1024, :].rearrange(
                "(s p) d -> p s d", p=128)
            nc.sync.dma_start(xld[g][:].rearrange("p (s d) -> p s d", s=8), src)

        def norm_group(g):
            """Row sum-of-squares, Newton rsqrt (*16, bf16), diag tiles."""
            for q in range(8):
                t = 8 * g + q
                jb = junkp.tile([128, D], BF16, tag="nj", name="nj")
                nc.vector.scalar_tensor_tensor(
                    jb[:], xn(t), 1.0, xn(t),
                    op0=ALU.mult, op1=ALU.mult, accum_out=ss[:, t:t + 1])
            sl = slice(8 * g, 8 * g + 8)
            nc.vector.tensor_scalar(iu[:, sl], ss[:, sl].bitcast(I32), 1, None,
                                    op0=ALU.arith_shift_right)
            nc.vector.tensor_scalar(iv[:, sl], iu[:, sl], -1, 0x5F3759DF,
                                    op0=ALU.mult, op1=ALU.add)
            y = iv[:, sl].bitcast(F32)
            for it in range(3):
                nc.vector.tensor_mul(nt_t[:, sl], y, y)
                nc.vector.tensor_mul(nt_t[:, sl], nt_t[:, sl], ss[:, sl])
                nc.vector.tensor_scalar(nt_t[:, sl], nt_t[:, sl], -0.5, 1.5,
                                        op0=ALU.mult, op1=ALU.add)
                out_y = invn[:, sl] if it == 2 else y
                nc.vector.tensor_mul(out_y, y, nt_t[:, sl])
            nc.vector.tensor_scalar(invn16[:, sl], invn[:, sl], SCALE, None,
                                    op0=ALU.mult)
            # dts[:, t*128+c] = (p==c) * invn16[p, t] for the group's tiles
            nc.vector.tensor_tensor(
                dts[:, 8 * g * 128:(8 * g + 8) * 128].rearrange(
                    "p (s c) -> p s c", s=8),
                eye16[:, None, :].to_broadcast([128, 8, 128]),
                invn16[:, sl, None].to_broadcast([128, 8, 128]),
                op=ALU.mult)

        def transpose_pair(j):
            """xt[j][:, k*1024 + tt*128 + r] = xn(8j+tt)[r, k*128+d] * invn16*16."""
            for kk in range(2):
                ps = psm.tile([128, 2048], F32, tag="ps", name=f"tf{j}_{kk}")
                for tt in range(8):
                    t = 8 * j + tt
                    for ks in range(2):
                        k = 2 * kk + ks
                        nc.tensor.matmul(
                            ps[:, ks * 1024 + tt * 128:ks * 1024 + (tt + 1) * 128],
                            lhsT=xn(t)[:, k * 128:(k + 1) * 128],
                            rhs=dts[:, t * 128:(t + 1) * 128],
                            start=True, stop=True)
                dst = xt[j][:, kk * 2048:(kk + 1) * 2048]
                if kk == 0:
                    nc.scalar.copy(dst, ps[:])
                else:
                    nc.vector.tensor_copy(dst, ps[:])

        def dr_ap(j, kk, lo, w):
            """[128, 2, w] DoubleRow AP over xt[j], k-slices {2kk, 2kk+1}."""
            return xt[j][:, kk * 2048:(kk + 1) * 2048].rearrange(
                "p (two c) -> p two c", two=2)[:, :, lo:lo + w]

        def main_fill(m, pgi):
            pg = PGROUPS[pgi]
            w = 1024 * len(pg)
            ps = psm.tile([128, 2048], F32, tag="ps", name=f"mf{m}_{pgi}")
            # kk outer: one stationary serves all (j, h) of this fill, so
            # consecutive LDWEIGHTS are identical and can collapse/hide.
            for kk in range(2):
                for ji, j in enumerate(pg):
                    for h in range(2):
                        nc.tensor.matmul(
                            ps[:, ji * 1024 + h * 512:ji * 1024 + (h + 1) * 512],
                            lhsT=dr_ap(0, kk, m * 128, 128),
                            rhs=dr_ap(j, kk, h * 512, 512),
                            start=(kk == 0), stop=(kk == 1),
                            perf_mode=DR, skip_group_check=True)
            # pre-exp extracts from PSUM (exact fp32): self-diag / positive
            if pgi == 0 or pgi == 2:
                tgt = sdiag if pgi == 0 else spos
                junk = junkp.tile([128, 128], F32, tag="xj", name="xj")
                nc.vector.scalar_tensor_tensor(
                    junk[:], ps[:, m * 128:(m + 1) * 128], 1.0, eye32[:],
                    op0=ALU.mult, op1=ALU.mult, accum_out=tgt[:, m:m + 1])
            eo = eop.tile([128, 2048], BF16, tag="eo", name=f"eo{m}_{pgi}")
            nc.scalar.activation(eo[:, :w], ps[:, :w], ACTF.Exp, scale=EXPS,
                                 accum_out=rs[:, m * 3 + pgi:m * 3 + pgi + 1])
            for ji, j in enumerate(pg):
                if j in (1, 2, 3):
                    src = eo[:, ji * 1024:(ji + 1) * 1024]
                    if m == 0:
                        nc.vector.tensor_copy(colacc[j - 1][:], src)
                    else:
                        nc.vector.tensor_add(colacc[j - 1][:],
                                             colacc[j - 1][:], src)

        # ---- emission schedule (pipelined) ----
        norm_group(0)
        norm_group(1)
        transpose_pair(0)
        transpose_pair(1)
        for m in range(3):
            main_fill(m, 0)
        norm_group(2)
        transpose_pair(2)
        for m in range(3, 6):
            main_fill(m, 0)
        norm_group(3)
        transpose_pair(3)
        for m in range(6, 8):
            main_fill(m, 0)
        for m in range(3):
            main_fill(m, 1)
        norm_group(4)
        transpose_pair(4)
        for m in range(3, 8):
            main_fill(m, 1)
        for m in range(8):
            main_fill(m, 2)

        # ---- finalize ----
        nc.vector.tensor_reduce(rowsum[:], rs[:].rearrange("p (m q) -> p m q", m=MB),
                                axis=AX.X, op=ALU.add)
        nc.scalar.activation(diagexp[:], sdiag[:], ACTF.Exp, scale=EXPS)
        nc.scalar.activation(numv[:], spos[:], ACTF.Exp, scale=EXPS)
        nc.vector.tensor_sub(dent[:], rowsum[:], diagexp[:])
        nc.sync.dma_start(numer_out, numv[:])
        nc.sync.dma_start(denom_out, dent[:])
        for j in range(3):
            nc.sync.dma_start(colsum_out[:, j * RPC:(j + 1) * RPC], colacc[j][:])

    nc.finalize()
    return nc


@functools.lru_cache(maxsize=1)
def _get_nc():
    return _build()


def _in_maps(x):
    x = np.asarray(x)
    assert x.shape == (N, D)
    xb = np.asarray(x, dtype=np.float32).astype(ml_dtypes.bfloat16)
    eye16 = np.eye(128, dtype=ml_dtypes.bfloat16)
    eye32 = np.eye(128, dtype=np.float32)
    return [
        {"x": np.ascontiguousarray(np.roll(xb, -c * RPC, axis=0)[:ROWS_IN]),
         "eye16": eye16, "eye32": eye32}
        for c in range(NCORES)
    ]


def _run(x, **run_kwargs):
    from concourse.bass_utils import run_bass_kernel_spmd

    nc = _get_nc()
    return run_bass_kernel_spmd(nc, _in_maps(x), list(range(NCORES)),
                                **run_kwargs)


def _loss_from_results(results):
    num = np.concatenate(
        [results[c]["numer"].T.reshape(-1).astype(np.float64)
         for c in range(NCORES)])
    den_own = [results[c]["denom"].T.reshape(-1).astype(np.float64)
               for c in range(NCORES)]
    # colsum[c][:, (j-1)*1024 + cl].sum over partitions = partial denom for
    # global row (c+j)*1024 + cl, j in {1,2,3}
    cs = [np.asarray(results[c]["colsum"], dtype=np.float64)
          for c in range(NCORES)]
    den = np.concatenate([
        den_own[b] + sum(
            cs[(b - j) % NCORES][:, (j - 1) * RPC:j * RPC].sum(axis=0)
            for j in range(1, 4))
        for b in range(NCORES)])
    loss = -np.sum(np.log(num / den)) / N
    return np.float32(loss)


def kernel(x):
    res = _run(x)
    return _loss_from_results(res.results)
